# revision 25
# baseline (speedup 1.0000x reference)
"""MoE gate routing kernel for Trainium2 (8 NeuronCores, Bass/Tile).

Computes, for hidden_states [4, 4096, 7168] (f32), gate kernel [7168, 256],
e_score_correction_bias [256]:
    logits = x @ W ; scores = sigmoid(logits) + bias
    grouped top-2-sum -> top-4 groups of 8 -> mask -> top-8 experts
    weights = 2.5 * topk_vals / sum(topk_vals)
Returns (topk_idx int32 [16384, 8], topk_weight f32 [16384, 8]).

Sharding: tokens split evenly across 8 cores (2048 tokens/core); W + bias
replicated. No cross-core communication.
"""
import sys

sys.path.insert(0, "/opt/trn_rl_repo")

import numpy as np

import concourse.bass as bass  # noqa: F401  (engine types referenced via nc)
import concourse.mybir as mybir
import concourse.tile as tile
from concourse import bacc
from concourse.bass_utils import run_bass_kernel_spmd
from concourse.masks import make_identity

# Problem constants (hardcoded per contract)
H = 7168
E = 256
N_CORES = 8
T_FULL = 4 * 4096           # 16384 tokens
T_C = T_FULL // N_CORES     # 2048 tokens per core
P = 128
KT = H // P                 # 56 contraction tiles
TT = T_C // P               # 16 token tiles per core
GROUP = 4                   # k-tiles per transpose psum tile
NG = KT // GROUP            # 14
PIPE = 5                    # transpose groups in flight ahead of matmuls transpose groups
XCH = 7 * P                 # x DMA chunk width (7 k-tiles); 8 chunks per t-tile
N_GROUP = 8
TOPK_GROUP = 4
TOP_K = 8
EPG = E // N_GROUP          # 32 experts per group
SCALE = 2.5

f32 = mybir.dt.float32
u32 = mybir.dt.uint32

_CACHED_NC = None


def _build_nc():
    nc = bacc.Bacc("TRN2", target_bir_lowering=False, debug=False)
    x = nc.dram_tensor("x", [T_C, H], f32, kind="ExternalInput")
    w = nc.dram_tensor("w", [H, E], f32, kind="ExternalInput")
    b = nc.dram_tensor("b", [E], f32, kind="ExternalInput")
    idx_out = nc.dram_tensor("idx_out", [T_C, TOP_K], u32, kind="ExternalOutput")
    wt_out = nc.dram_tensor("wt_out", [T_C, TOP_K], f32, kind="ExternalOutput")

    w_kpe = w.ap().rearrange("(ko p) e -> p ko e", p=P)  # [128, 56, 256]
    # p-major token mapping: tile t holds tokens {p*TT + t}. Makes the final
    # output DMA per-partition lines contiguous (TT*8 elems = 512B).
    x_tp = x.ap().rearrange("(p t) h -> p t h", t=TT)    # [128, 16, 7168]
    idx_tp = idx_out.ap().rearrange("(p t) k -> p t k", t=TT)
    wt_tp = wt_out.ap().rearrange("(p t) k -> p t k", t=TT)

    with tile.TileContext(nc) as tc:
        with (
            tc.tile_pool(name="const", bufs=1) as cpool,
            tc.tile_pool(name="xp", bufs=2) as x_pool,
            tc.tile_pool(name="xt", bufs=7) as xt_pool,
            tc.tile_pool(name="sc", bufs=2) as sc_pool,
            tc.tile_pool(name="tk", bufs=2) as tk_pool,
            tc.tile_pool(name="outp", bufs=1) as out_pool,
            tc.tile_pool(name="ps_l", bufs=2, space="PSUM") as ps_logits,
            tc.tile_pool(name="ps_t", bufs=6, space="PSUM") as ps_tr,
        ):
            ident = cpool.tile([P, P], f32)
            make_identity(nc, ident)

            # DMA order matters: x tile-0 chunks first (PE starts transposing
            # as soon as the first lands), then W streaming in behind
            # (consumed k-tile by k-tile by the matmuls), then bias (first
            # needed by the tile-0 sigmoid). Tile 0's first chunk is split so
            # the very first transpose starts earlier. 8 chunks/tile keep
            # each x-slot's steady-state DMAs on a fixed HWDGE queue.
            x_sb0 = x_pool.tile([P, H], f32, tag="x_sb")
            w_sb = cpool.tile([P, KT, E], f32)

            def x0c(c):
                sl = slice(c * XCH, (c + 1) * XCH)
                nc.sync.dma_start(x_sb0[:, sl], x_tp[:, 0, sl])

            def wblk(wb):
                nc.sync.dma_start(w_sb[:, wb : wb + 8], w_kpe[:, wb : wb + 8])

            # near-EDF interleave: first x chunks feed the transposes, W block
            # b feeds matmul groups 2b..2b+1
            for c in (0, 1, 2, 3):
                x0c(c)
            wblk(0)
            x0c(4); x0c(5)
            wblk(8)
            x0c(6); x0c(7)
            for wb in range(16, KT, 8):
                wblk(wb)

            bias_sb = cpool.tile([P, E], f32)
            nc.sync.dma_start(bias_sb, b.ap().unsqueeze(0).partition_broadcast(P))

            idx_acc = out_pool.tile([P, TT, TOP_K], u32)
            wt_acc = out_pool.tile([P, TT, TOP_K], f32)

            def topk_chain(t, logits):
                """Fused sigmoid+bias+grouped-top-k for one 128-token tile."""
                scores = sc_pool.tile([P, E], f32, tag="scores")
                nc.scalar.activation(
                    out=scores, in_=logits, func=mybir.ActivationFunctionType.Sigmoid
                )
                nc.vector.tensor_add(scores, scores, bias_sb)

                gmax8 = tk_pool.tile([P, N_GROUP, 8], f32, tag="gmax8")
                for g in range(N_GROUP):
                    nc.vector.max(out=gmax8[:, g], in_=scores[:, g * EPG : (g + 1) * EPG])
                gsum = tk_pool.tile([P, N_GROUP], f32, tag="gsum")
                nc.vector.tensor_add(gsum, gmax8[:, :, 0], gmax8[:, :, 1])
                gs8 = tk_pool.tile([P, 8], f32, tag="gs8")
                nc.vector.max(out=gs8, in_=gsum)
                gmask = tk_pool.tile([P, N_GROUP], f32, tag="gmask")
                nc.vector.tensor_scalar(
                    out=gmask, in0=gsum,
                    scalar1=gs8[:, TOPK_GROUP - 1 : TOPK_GROUP], scalar2=None,
                    op0=mybir.AluOpType.is_ge,
                )
                masked = sc_pool.tile([P, E], f32, tag="masked")
                nc.vector.tensor_mul(
                    masked.rearrange("p (g j) -> p g j", g=N_GROUP),
                    scores.rearrange("p (g j) -> p g j", g=N_GROUP),
                    gmask.unsqueeze(2).to_broadcast([P, N_GROUP, EPG]),
                )
                vals8 = tk_pool.tile([P, 8], f32, tag="vals8")
                nc.vector.max(out=vals8, in_=masked)
                nc.vector.max_index(out=idx_acc[:, t], in_max=vals8, in_values=masked)
                denom = tk_pool.tile([P, 1], f32, tag="denom")
                nc.vector.reduce_sum(out=denom, in_=vals8, axis=mybir.AxisListType.X)
                inv = tk_pool.tile([P, 1], f32, tag="inv")
                nc.vector.reciprocal(inv, denom)
                nc.vector.tensor_scalar(
                    out=wt_acc[:, t], in0=vals8,
                    scalar1=inv[:, 0:1], scalar2=SCALE,
                    op0=mybir.AluOpType.mult, op1=mybir.AluOpType.mult,
                )
                if t == TT - 2:
                    # bulk of the output leaves while the last tile computes
                    nc.sync.dma_start(idx_tp[:, : TT - 1], idx_acc[:, : TT - 1])
                    nc.sync.dma_start(wt_tp[:, : TT - 1], wt_acc[:, : TT - 1])
                elif t == TT - 1:
                    nc.sync.dma_start(idx_tp[:, TT - 1 :], idx_acc[:, TT - 1 :])
                    nc.sync.dma_start(wt_tp[:, TT - 1 :], wt_acc[:, TT - 1 :])

            # Flat software pipeline over all (tile, group) pairs: transposes
            # run PIPE groups ahead of the matmuls that consume them, with no
            # barrier at t-tile boundaries.
            TOTAL = TT * NG
            x_tiles = {0: x_sb0}
            xt_flat = [None] * TOTAL
            logits_of = {}

            def emit_mms(Gf):
                t, g = divmod(Gf, NG)
                xt4 = xt_flat[Gf]
                logits = logits_of[t]
                for i in range(GROUP):
                    kt = g * GROUP + i
                    nc.tensor.matmul(
                        logits,
                        xt4[:, i * P : (i + 1) * P],
                        w_sb[:, kt, :],
                        start=(kt == 0),
                        stop=(kt == KT - 1),
                    )
                if g == NG - 1:
                    topk_chain(t, logits)

            for Gf in range(TOTAL + PIPE):
                if Gf < TOTAL:
                    t, g = divmod(Gf, NG)
                    if g == 0:
                        if t >= 1:
                            x_sb = x_pool.tile([P, H], f32, tag="x_sb")
                            nc.sync.dma_start(x_sb[:, : H // 2], x_tp[:, t, : H // 2])
                            nc.sync.dma_start(x_sb[:, H // 2 :], x_tp[:, t, H // 2 :])
                            x_tiles[t] = x_sb
                        logits = ps_logits.tile([P, E], f32, tag="logits")
                        logits_of[t] = logits
                    x_sb = x_tiles[t]
                    pst = ps_tr.tile([P, GROUP * P], f32, tag="pst")
                    for i in range(GROUP):
                        kt = g * GROUP + i
                        nc.tensor.transpose(
                            pst[:, i * P : (i + 1) * P],
                            x_sb[:, kt * P : (kt + 1) * P],
                            ident,
                        )
                    xt4 = xt_pool.tile([P, GROUP * P], f32, tag="xt4")
                    nc.scalar.copy(out=xt4, in_=pst)
                    xt_flat[Gf] = xt4
                if Gf >= PIPE:
                    emit_mms(Gf - PIPE)



    nc.compile()
    return nc


def get_nc():
    global _CACHED_NC
    if _CACHED_NC is None:
        _CACHED_NC = _build_nc()
    return _CACHED_NC


def run(hidden_states, kernel_w, bias, trace=False, trace_cores=None):
    """Internal entry that also exposes trace results for benchmarking."""
    x_full = np.ascontiguousarray(
        np.asarray(hidden_states, dtype=np.float32).reshape(T_FULL, H)
    )
    w_np = np.ascontiguousarray(np.asarray(kernel_w, dtype=np.float32))
    b_np = np.ascontiguousarray(np.asarray(bias, dtype=np.float32))

    nc = get_nc()
    in_maps = [
        {"x": x_full[c * T_C : (c + 1) * T_C], "w": w_np, "b": b_np}
        for c in range(N_CORES)
    ]
    kw = {}
    if trace:
        kw = dict(trace=True, trace_cores=trace_cores or [0])
    last_err = None
    for attempt in range(3):
        try:
            res = run_bass_kernel_spmd(nc, in_maps, core_ids=list(range(N_CORES)), **kw)
            break
        except Exception as e:  # transient NRT/axon device hiccups
            last_err = e
            if attempt == 2:
                raise
            import time as _time

            _time.sleep(15)
    else:
        raise last_err

    idx = np.concatenate([r["idx_out"] for r in res.results], axis=0).astype(np.int32)
    wt = np.concatenate([r["wt_out"] for r in res.results], axis=0)
    return (idx, wt), res


def kernel(hidden_states, kernel, e_score_correction_bias):
    (idx, wt), _ = run(hidden_states, kernel, e_score_correction_bias)
    return idx, wt
